# revision 1
# baseline (speedup 1.0000x reference)
"""Head-parallel MHA kernel for Trainium2 (8 NeuronCores).

Problem: pre-LN multi-head attention, B=2, S=2048, D=1024, H=16, HD=64, fp32.
Sharding: tensor-parallel over heads — core c owns heads (2c, 2c+1), i.e.
columns [128c, 128c+128) of Wq/Wk/Wv and rows [128c, 128c+128) of Wo.

Device math (bf16 matmuls, fp32 PSUM accumulation):
  zT = (xT - mu) * rstd          (LN; gamma/beta folded into weights on host)
  qT = Wq_c^T zT + bq_c ; kT likewise ; V = z Wv_c      (per-core slices)
  sT[k,q] = kT_h^T qT_h ;  es = exp(s/8 - 12)           (shift for fp range)
  ctxT_h (+ colsum via ones column in V) = V_h'^T es    (accumulated over k)
  outA/outB = (ctxT_h)^T Wo_h   per head, bf16 partials (UNNORMALIZED)
Host: out = x + sum_c sum_h out_h_c / colsum_h_c + (bo + (beta Wv + bv) Wo)
"""
import numpy as np
import ml_dtypes

import concourse.bass as bass
from concourse import bacc
import concourse.mybir as mybir
import concourse.tile as tile
from concourse.tile_rust import add_dep_helper
from concourse.bass_utils import run_bass_kernel_spmd

dt = mybir.dt
F32, BF16 = dt.float32, dt.bfloat16
BF = ml_dtypes.bfloat16
Alu = mybir.AluOpType
Act = mybir.ActivationFunctionType

B, S, D = 2, 2048, 1024
T = B * S            # 4096 tokens
DH = 128             # per-core head dims (2 heads x 64)
NKB = 16             # 128-wide k blocks per batch
QT = 1024            # q tile columns
NQT = S // QT        # q tiles per batch
EXP_SHIFT = -12.0
LN_EPS = 1e-5
N_CORES = 8

_CACHE = {}


def _build_nc():
    if "nc" in _CACHE:
        return _CACHE["nc"]
    nc = bacc.Bacc("TRN2", target_bir_lowering=False)
    xbf_d = nc.dram_tensor("xbf", [T, D], BF16, kind="ExternalInput")
    wq_d = nc.dram_tensor("wq", [D, DH], BF16, kind="ExternalInput")
    wk_d = nc.dram_tensor("wk", [D, DH], BF16, kind="ExternalInput")
    wv_d = nc.dram_tensor("wv", [D, DH], BF16, kind="ExternalInput")
    wo_d = nc.dram_tensor("wo", [DH, D], BF16, kind="ExternalInput")
    bq_d = nc.dram_tensor("bq", [DH, 1], F32, kind="ExternalInput")
    bk_d = nc.dram_tensor("bk", [DH, 1], F32, kind="ExternalInput")
    id_d = nc.dram_tensor("ident", [128, 128], BF16, kind="ExternalInput")

    outA_d = nc.dram_tensor("outA", [T, D], BF16, kind="ExternalOutput")
    outB_d = nc.dram_tensor("outB", [T, D], BF16, kind="ExternalOutput")
    cs_d = nc.dram_tensor("cs", [8, QT], F32, kind="ExternalOutput")

    mur_d = nc.dram_tensor("mur_scratch", [64, 128], BF16)  # internal
    mur2_d = nc.dram_tensor("mur2_scratch", [64, 128], BF16)  # internal
    trash_d = nc.dram_tensor("trash_scratch", [1, 16], BF16)  # internal

    with tile.TileContext(nc) as tc:
        _body(nc, tc, xbf_d, wq_d, wk_d, wv_d, wo_d, bq_d, bk_d, id_d,
              outA_d, outB_d, cs_d, mur_d, mur2_d, trash_d)
    nc.compile()
    _CACHE["nc"] = nc
    return nc


def _body(nc, tc, xbf_d, wq_d, wk_d, wv_d, wo_d, bq_d, bk_d, id_d,
          outA_d, outB_d, cs_d, mur_d, mur2_d, trash_d):
    import contextlib
    ctx = contextlib.ExitStack()
    const = ctx.enter_context(tc.tile_pool(name="const", bufs=1))
    xpool = ctx.enter_context(tc.tile_pool(name="xpool", bufs=4))
    spool = ctx.enter_context(tc.tile_pool(name="spool", bufs=4))
    espool = ctx.enter_context(tc.tile_pool(name="espool", bufs=3))
    opool = ctx.enter_context(tc.tile_pool(name="opool", bufs=2))
    psum = ctx.enter_context(tc.tile_pool(name="psum", bufs=1, space="PSUM"))

    trash = const.tile([1, 64], BF16, tag="trash")

    def fence(src_ap, n):
        # tiny gpsimd copy: makes the Pool engine observe src's producer sem
        # so the following store carries only its self-queue wait.
        return nc.gpsimd.tensor_copy(out=trash[0:1, n:n + 2], in_=src_ap)

    def after(dma_inst, fence_inst):
        add_dep_helper(dma_inst.ins, fence_inst.ins, sync=False,
                       reason="order dma after fence")

    # ---- Phase 0: xbar transposes of the raw input (no deps) + loads ----
    xT = const.tile([128, 8, T], BF16, tag="xT")       # becomes zT in place
    for dc in range(8):
        nc.sync.dma_start_transpose(out=xT[:, dc, :],
                                    in_=xbf_d[:, dc * 128:(dc + 1) * 128])

    tfences = [fence(xT[0:1, dc, 0:2], 16 + 2 * dc) for dc in range(8)]
    tfence = tfences[-1]
    wq_l = const.tile([128, 8, DH], BF16, tag="wq_l")
    wk_l = const.tile([128, 8, DH], BF16, tag="wk_l")
    wv_l = const.tile([128, 8, DH], BF16, tag="wv_l")
    wq_sb = const.tile([128, 8, DH], BF16, tag="wq")
    wk_sb = const.tile([128, 8, DH], BF16, tag="wk")
    wv_sb = const.tile([128, 8, DH], BF16, tag="wv")
    for dc in range(8):
        nc.gpsimd.dma_start(out=wq_sb[:, dc, :], in_=wq_d[dc * 128:(dc + 1) * 128, :])
        nc.gpsimd.dma_start(out=wk_sb[:, dc, :], in_=wk_d[dc * 128:(dc + 1) * 128, :])
        nc.gpsimd.dma_start(out=wv_sb[:, dc, :], in_=wv_d[dc * 128:(dc + 1) * 128, :])
    wo_sb = const.tile([128, D], BF16, tag="wo")
    nc.gpsimd.dma_start(out=wo_sb, in_=wo_d[:, :])
    bq_sb = const.tile([128, 1], F32, tag="bq")
    bk_sb = const.tile([128, 1], F32, tag="bk")
    nc.gpsimd.dma_start(out=bq_sb, in_=bq_d[:, :])
    nc.gpsimd.dma_start(out=bk_sb, in_=bk_d[:, :])
    id_sb = const.tile([128, 128], BF16, tag="ident")
    nc.gpsimd.dma_start(out=id_sb, in_=id_d[:, :])
    eps_sb = const.tile([128, 1], F32, tag="eps")
    nc.vector.memset(eps_sb, LN_EPS)
    ebias = const.tile([128, 1], F32, tag="ebias")
    nc.vector.memset(ebias, EXP_SHIFT)


    # ---- Phase 1: LN stats (token-major); x in 4 big tiles (no slot reuse)
    mur_all = const.tile([128, 64], BF16, tag="mur_all")  # cols 0:32 mu, 32:64 r
    for g in range(8):
        xg = xpool.tile([128, 4, D], BF16, tag="xt", bufs=8, name=f"xg{g}")
        nc.gpsimd.dma_start(
            out=xg, in_=xbf_d[g * 512:(g + 1) * 512, :].rearrange(
                "(a b) c -> b a c", a=4))
        for j in range(4):
            ti = g * 4 + j
            st = spool.tile([128, 2, 6], F32, tag="st", name=f"st{ti}")
            nc.vector.bn_stats(out=st[:, 0, :], in_=xg[:, j, 0:512])
            nc.vector.bn_stats(out=st[:, 1, :], in_=xg[:, j, 512:1024])
            mv = spool.tile([128, 2], F32, tag="mv", name=f"mv{ti}")
            nc.vector.bn_aggr(out=mv, in_=st)
            nc.vector.tensor_copy(out=mur_all[:, ti:ti + 1], in_=mv[:, 0:1])
            std = spool.tile([128, 1], F32, tag="std", name=f"std{ti}")
            nc.scalar.activation(out=std, in_=mv[:, 1:2], func=Act.Sqrt,
                                 bias=eps_sb, scale=1.0)
            rstd = spool.tile([128, 1], F32, tag="rstd", name=f"rstd{ti}")
            nc.vector.reciprocal(out=rstd, in_=std)
            nc.vector.tensor_copy(out=mur_all[:, 32 + ti:33 + ti], in_=rstd)

    # PE transpose [mu|r] -> rows; stage to DRAM; broadcast back
    nc.tensor.ldweights(id_sb[0:1, 0:2])      # absorb id DMA sem on PE
    murps = psum.tile([64, 128], BF16, tag="ctxA")
    nc.tensor.matmul(murps, lhsT=mur_all, rhs=id_sb, start=True, stop=True,
                     is_transpose=True)
    mur_rows = spool.tile([64, 128], BF16, tag="mur_rows")
    nc.vector.tensor_copy(out=mur_rows, in_=murps)
    nc.gpsimd.dma_start(out=mur_d[:, :], in_=mur_rows)
    MU = const.tile([128, T], BF16, tag="qT")
    R = const.tile([128, T], BF16, tag="kT")
    for q in range(4):
        nc.gpsimd.dma_start(out=MU[:, q * 1024:(q + 1) * 1024],
                            in_=bass.AP(tensor=mur_d, offset=q * 1024,
                                        ap=[[0, 128], [1, 1024]]))
        nc.gpsimd.dma_start(out=R[:, q * 1024:(q + 1) * 1024],
                            in_=bass.AP(tensor=mur_d, offset=T + q * 1024,
                                        ap=[[0, 128], [1, 1024]]))

    # zT = (xT - MU) * R in place, per d-chunk and half for pipelining
    for dc in range(8):
        for hh in range(2):
            sl = slice(hh * 2048, (hh + 1) * 2048)
            nc.vector.tensor_sub(out=xT[:, dc, sl], in0=xT[:, dc, sl], in1=MU[:, sl])
            nc.vector.tensor_mul(out=xT[:, dc, sl], in0=xT[:, dc, sl], in1=R[:, sl])
    zT = xT

    # ---- Phase 2: QKV projections ----
    for dc in range(8):  # absorb weight-load DMA sems on PE
        nc.tensor.ldweights(wq_sb[0:1, dc, 0:2])
        nc.tensor.ldweights(wk_sb[0:1, dc, 0:2])
        nc.tensor.ldweights(wv_sb[0:1, dc, 0:2])
    nc.tensor.ldweights(wo_sb[0:1, 0:2])
    qT = const.tile([128, T], BF16, tag="qT")
    kT = const.tile([128, T], BF16, tag="kT")
    v_sb = const.tile([128, 32, 130], BF16, tag="v")
    nc.vector.memset(v_sb[:, :, 64:65], 1.0)
    nc.vector.memset(v_sb[:, :, 129:130], 1.0)

    for tsl in range(8):  # 512-token slices
        cols = slice(tsl * 512, (tsl + 1) * 512)
        for name, wsb, bsb, dst in (("q", wq_sb, bq_sb, qT), ("k", wk_sb, bk_sb, kT)):
            ab = None
            if tsl > 0:
                ab = nc.tensor.ldweights(dst[0:1, (tsl - 1) * 512:(tsl - 1) * 512 + 2])
            ps = psum.tile([128, 512], F32, tag="sA" if name == "q" else "sB",
                           name=f"ps{name}{tsl}")
            for dc in range(8):
                mm = nc.tensor.matmul(ps, lhsT=wsb[:, dc, :], rhs=zT[:, dc, cols],
                                      start=(dc == 0), stop=(dc == 7))
                if dc == 0 and ab is not None:
                    after(mm, ab)
            nc.vector.tensor_scalar(out=dst[:, cols], in0=ps, scalar1=bsb,
                                    scalar2=None, op0=Alu.add)
    for ck in range(32):  # V: 128-token chunks, natural layout
        tok = slice(ck * 128, (ck + 1) * 128)
        ab = None
        if ck >= 2:
            ab = nc.tensor.ldweights(v_sb[0:1, ck - 2, 0:2])
        psv = psum.tile([128, 128], F32, tag="ctxA" if ck % 2 == 0 else "ctxB",
                        name=f"psv{ck}")
        for dc in range(8):
            mm = nc.tensor.matmul(psv, lhsT=zT[:, dc, tok], rhs=wv_sb[:, dc, :],
                                  start=(dc == 0), stop=(dc == 7))
            if dc == 0 and ab is not None:
                after(mm, ab)
        nc.vector.tensor_copy(out=v_sb[:, ck, 0:64], in_=psv[:, 0:64])
        nc.vector.tensor_copy(out=v_sb[:, ck, 65:129], in_=psv[:, 64:128])

    # ---- Phase 3: attention ----
    nc.tensor.ldweights(qT[0:1, T - 2:T])
    nc.tensor.ldweights(kT[0:1, T - 2:T])
    nc.tensor.ldweights(v_sb[0:1, 30, 0:2])
    nc.tensor.ldweights(v_sb[0:1, 31, 0:2])
    ctxT = const.tile([128, T], BF16, tag="ctxT")

    for b in range(B):
        for qt in range(NQT):
            seg = b * NQT + qt  # 0..3
            qcols = slice(b * S + qt * QT, b * S + (qt + 1) * QT)
            ctxps = {}
            for h, tag in ((0, "ctxA"), (1, "ctxB")):
                ctxps[h] = psum.tile([65, QT], F32, tag=tag, name=f"ctx{seg}h{h}")
            for kb in range(NKB):
                ck = b * NKB + kb
                kcols = slice(b * S + kb * 128, b * S + (kb + 1) * 128)
                for h, stag in ((0, "sA"), (1, "sB")):
                    hp = slice(64 * h, 64 * (h + 1))
                    sps = psum.tile([128, QT], F32, tag=stag, name=f"s{seg}k{kb}h{h}")
                    for half in range(2):
                        qh = slice(qcols.start + half * 512,
                                   qcols.start + (half + 1) * 512)
                        nc.tensor.matmul(sps[:, half * 512:(half + 1) * 512],
                                         lhsT=kT[hp, kcols], rhs=qT[hp, qh],
                                         start=True, stop=True)
                    es = espool.tile([128, QT], BF16, tag=f"es{h}",
                                     name=f"es{seg}k{kb}h{h}")
                    nc.scalar.activation(out=es, in_=sps, func=Act.Exp,
                                         bias=ebias, scale=0.125)
                    for half in range(2):
                        hs = slice(half * 512, (half + 1) * 512)
                        nc.tensor.matmul(ctxps[h][:, hs],
                                         lhsT=v_sb[:, ck, h * 65:(h + 1) * 65],
                                         rhs=es[:, hs],
                                         start=(kb == 0), stop=(kb == NKB - 1))
            # drain ctx + colsum on the Scalar engine so the psum slot release
            # merges into the ACT waits PE already carries
            for h in range(2):
                hp = slice(64 * h, 64 * (h + 1))
                nc.vector.tensor_copy(out=ctxT[hp, qcols], in_=ctxps[h][0:64, :])
                csrow = espool.tile([65, QT], F32, tag="cs", bufs=2,
                                    name=f"cs{seg}h{h}")
                nc.vector.tensor_copy(out=csrow[64:65, :], in_=ctxps[h][64:65, :])
                cf = fence(csrow[64:65, 0:2], 2)
                sd = nc.gpsimd.dma_start(out=cs_d[seg * 2 + h:seg * 2 + h + 1, :],
                                         in_=csrow[64:65, :])
                after(sd, cf)
            # out_proj for this segment, overlapped with the next segment's
            # attention: reuse the just-drained ctx psum slots
            for j in range(8):
                blk = (qcols.start // 128) + j
                tok = slice(blk * 128, (blk + 1) * 128)
                for h, (od, tagps) in enumerate(((outA_d, "ctxA"), (outB_d, "ctxB"))):
                    hp = slice(64 * h, 64 * (h + 1))
                    ops = psum.tile([128, D], F32, tag=tagps, name=f"ops{blk}h{h}")
                    for half in range(2):
                        ocols = slice(half * 512, (half + 1) * 512)
                        nc.tensor.matmul(ops[:, ocols], lhsT=ctxT[hp, tok],
                                         rhs=wo_sb[hp, ocols], start=True, stop=True)
                    osb = opool.tile([128, D], BF16, tag=f"osb{h}",
                                     name=f"osb{blk}h{h}")
                    nc.vector.tensor_copy(out=osb, in_=ops)
                    of = fence(osb[0:1, 0:2], 4 + 2 * h)
                    after(nc.gpsimd.dma_start(out=od[tok, :], in_=osb), of)


    ctx.close()


def kernel(x, Wq, bq, Wk, bk, Wv, bv, Wo, bo, ln_gamma, ln_beta):
    x = np.asarray(x, np.float32)
    Wq, Wk, Wv, Wo = (np.asarray(w, np.float32) for w in (Wq, Wk, Wv, Wo))
    bq, bk, bv, bo = (np.asarray(v, np.float32) for v in (bq, bk, bv, bo))
    g, be = np.asarray(ln_gamma, np.float32), np.asarray(ln_beta, np.float32)

    x2 = x.reshape(T, D)
    xbf = x2.astype(BF)
    Wq_e, Wk_e, Wv_e = g[:, None] * Wq, g[:, None] * Wk, g[:, None] * Wv
    bq_e, bk_e = be @ Wq + bq, be @ Wk + bk
    host_const = (bo + (be @ Wv + bv) @ Wo).astype(np.float32)
    ident = np.eye(128, dtype=np.float32).astype(BF)

    nc = _build_nc()
    in_maps = []
    for c in range(N_CORES):
        sl = slice(128 * c, 128 * (c + 1))
        in_maps.append({
            "xbf": xbf,
            "wq": Wq_e[:, sl].astype(BF),
            "wk": Wk_e[:, sl].astype(BF),
            "wv": Wv_e[:, sl].astype(BF),
            "wo": Wo[sl, :].astype(BF),
            "bq": bq_e[sl].reshape(DH, 1).astype(np.float32),
            "bk": bk_e[sl].reshape(DH, 1).astype(np.float32),
            "ident": ident,
        })
    res = run_bass_kernel_spmd(nc, in_maps, list(range(N_CORES))).results

    acc = np.zeros((T, D), np.float32)
    for c in range(N_CORES):
        cs = res[c]["cs"].astype(np.float32)  # [8, QT]: row = seg*2 + h
        for h, key in ((0, "outA"), (1, "outB")):
            o = res[c][key].astype(np.float32)  # [T, D]
            rc = np.empty(T, np.float32)
            for b in range(B):
                for qt in range(NQT):
                    seg = b * NQT + qt
                    rc[b * S + qt * QT:b * S + (qt + 1) * QT] = cs[seg * 2 + h]
            acc += o / rc[:, None]
    out = x2 + acc + host_const[None, :]
    return out.reshape(B, S, D).astype(np.float32)



# revision 9
# speedup vs baseline: 7.4214x; 7.4214x over previous
"""Head-parallel MHA kernel for Trainium2 (8 NeuronCores), device collectives.

Problem: pre-LN multi-head attention, B=2, S=2048, D=1024, H=16, HD=64, fp32.
Sharding: tensor-parallel over heads — core c owns heads (2c, 2c+1), i.e.
columns [128c, 128c+128) of Wq/Wk/Wv and rows [128c, 128c+128) of Wo.

Transfer-minimizing layout (wall-clock is dominated by the axon tunnel):
  up:   x token-slice [512, D] bf16 per core (AllGather on device to full x)
        + per-core weight slices (~1MB/core)
  down: final out token-slice [512, D] bf16 per core (device ReduceScatter
        of the fp32 out_proj partials across cores)

Device math (bf16 matmuls, fp32 PSUM accumulation):
  zT = (xT - mu) * rstd          (LN; gamma/beta folded into weights on host)
  qT = Wq_c^T zT + bq_c ; kT likewise ; V = z Wv_c      (per-core slices)
  sT[k,q] = kT_h^T qT_h ;  es = exp(s/8 - 12)           (shift for fp range)
  ctxT_h (+ colsum via ones column in V) = V_h'^T es    (accumulated over k)
  out_h = (ctxT_h)^T Wo_h * (1/colsum_h)   per head, fp32  (normalized)
  partial = out_h0 + out_h1 -> ReduceScatter(add) -> bf16 slice out
Host: out = x + concat(slices) + (bo + (beta Wv + bv) Wo)
"""
import numpy as np
import ml_dtypes

import concourse.bass as bass
from concourse import bacc
import concourse.mybir as mybir
import concourse.tile as tile
from concourse.tile_rust import add_dep_helper
from concourse.bass_utils import run_bass_kernel_spmd

dt = mybir.dt
F32, BF16 = dt.float32, dt.bfloat16
BF = ml_dtypes.bfloat16
Alu = mybir.AluOpType
Act = mybir.ActivationFunctionType

B, S, D = 2, 2048, 1024
T = B * S            # 4096 tokens
TS = T // 8          # 512-token slice per core
DH = 128             # per-core head dims (2 heads x 64)
NKB = 16             # 128-wide k blocks per batch
QT = 1024            # q tile columns
NQT = S // QT        # q tiles per batch
EXP_SHIFT = -12.0
LN_EPS = 1e-5
N_CORES = 8
GROUPS = [list(range(N_CORES))]

_CACHE = {}


def _build_nc():
    if "nc" in _CACHE:
        return _CACHE["nc"]
    nc = bacc.Bacc("TRN2", target_bir_lowering=False)
    xsl_d = nc.dram_tensor("xsl", [TS, D], BF16, kind="ExternalInput")
    wq_d = nc.dram_tensor("wq", [D, DH], BF16, kind="ExternalInput")
    wk_d = nc.dram_tensor("wk", [D, DH], BF16, kind="ExternalInput")
    wv_d = nc.dram_tensor("wv", [D, DH], BF16, kind="ExternalInput")
    wo_d = nc.dram_tensor("wo", [DH, D], BF16, kind="ExternalInput")
    bq_d = nc.dram_tensor("bq", [DH, 1], F32, kind="ExternalInput")
    bk_d = nc.dram_tensor("bk", [DH, 1], F32, kind="ExternalInput")
    id_d = nc.dram_tensor("ident", [128, 128], BF16, kind="ExternalInput")

    out_d = nc.dram_tensor("out", [TS, D], BF16, kind="ExternalOutput")

    mur_d = nc.dram_tensor("mur_scratch", [64, 128], BF16)  # internal

    with tile.TileContext(nc) as tc:
        _body(nc, tc, xsl_d, wq_d, wk_d, wv_d, wo_d, bq_d, bk_d, id_d,
              out_d, mur_d)
    nc.compile()
    _CACHE["nc"] = nc
    return nc


def _body(nc, tc, xsl_d, wq_d, wk_d, wv_d, wo_d, bq_d, bk_d, id_d,
          out_d, mur_d):
    import contextlib
    ctx = contextlib.ExitStack()
    const = ctx.enter_context(tc.tile_pool(name="const", bufs=1))
    xpool = ctx.enter_context(tc.tile_pool(name="xpool", bufs=4))
    spool = ctx.enter_context(tc.tile_pool(name="spool", bufs=4))
    espool = ctx.enter_context(tc.tile_pool(name="espool", bufs=3))
    opool = ctx.enter_context(tc.tile_pool(name="opool", bufs=2))
    psum = ctx.enter_context(tc.tile_pool(name="psum", bufs=1, space="PSUM"))
    dram = ctx.enter_context(tc.tile_pool(name="dram", bufs=1, space="DRAM"))

    trash = const.tile([1, 64], BF16, tag="trash")

    def fence(src_ap, n):
        # tiny gpsimd copy: makes the Pool engine observe src's producer sem
        # so the following store carries only its self-queue wait.
        return nc.gpsimd.tensor_copy(out=trash[0:1, n:n + 2], in_=src_ap)

    def after(dma_inst, fence_inst):
        add_dep_helper(dma_inst.ins, fence_inst.ins, sync=False,
                       reason="order dma after fence")

    # ---- Phase -1: AllGather the full x from per-core token slices ----
    xin_db = dram.tile([TS, D], BF16)
    xg_db = dram.tile([T, D], BF16)
    part_db = dram.tile([T, D], F32)
    rs_db = dram.tile([TS, D], F32)
    nc.gpsimd.dma_start(out=xin_db[:], in_=xsl_d[:, :])
    nc.gpsimd.collective_compute(
        "AllGather", Alu.bypass, replica_groups=GROUPS,
        ins=[xin_db.opt()], outs=[xg_db.opt()])

    # ---- Phase 0: xbar transposes of the gathered input + loads ----
    xT = const.tile([128, 8, T], BF16, tag="xT")       # becomes zT in place
    for dc in range(8):
        nc.sync.dma_start_transpose(out=xT[:, dc, :],
                                    in_=xg_db[:, dc * 128:(dc + 1) * 128])

    tfences = [fence(xT[0:1, dc, 0:2], 16 + 2 * dc) for dc in range(8)]
    wq_sb = const.tile([128, 8, DH], BF16, tag="wq")
    wk_sb = const.tile([128, 8, DH], BF16, tag="wk")
    wv_sb = const.tile([128, 8, DH], BF16, tag="wv")
    for dc in range(8):
        nc.gpsimd.dma_start(out=wq_sb[:, dc, :], in_=wq_d[dc * 128:(dc + 1) * 128, :])
        nc.gpsimd.dma_start(out=wk_sb[:, dc, :], in_=wk_d[dc * 128:(dc + 1) * 128, :])
        nc.gpsimd.dma_start(out=wv_sb[:, dc, :], in_=wv_d[dc * 128:(dc + 1) * 128, :])
    wo_sb = const.tile([128, D], BF16, tag="wo")
    nc.gpsimd.dma_start(out=wo_sb, in_=wo_d[:, :])
    bq_sb = const.tile([128, 1], F32, tag="bq")
    bk_sb = const.tile([128, 1], F32, tag="bk")
    nc.gpsimd.dma_start(out=bq_sb, in_=bq_d[:, :])
    nc.gpsimd.dma_start(out=bk_sb, in_=bk_d[:, :])
    id_sb = const.tile([128, 128], BF16, tag="ident")
    nc.gpsimd.dma_start(out=id_sb, in_=id_d[:, :])
    eps_sb = const.tile([128, 1], F32, tag="eps")
    nc.vector.memset(eps_sb, LN_EPS)
    ebias = const.tile([128, 1], F32, tag="ebias")
    nc.vector.memset(ebias, EXP_SHIFT)

    # ---- Phase 1: LN stats (token-major); x in 8 big tiles (no slot reuse)
    mur_all = const.tile([128, 64], BF16, tag="mur_all")  # cols 0:32 mu, 32:64 r
    for g in range(8):
        xg = xpool.tile([128, 4, D], BF16, tag="xt", bufs=8, name=f"xg{g}")
        nc.gpsimd.dma_start(
            out=xg, in_=xg_db[g * 512:(g + 1) * 512, :].rearrange(
                "(a b) c -> b a c", a=4))
        for j in range(4):
            ti = g * 4 + j
            st = spool.tile([128, 2, 6], F32, tag="st", name=f"st{ti}")
            nc.vector.bn_stats(out=st[:, 0, :], in_=xg[:, j, 0:512])
            nc.vector.bn_stats(out=st[:, 1, :], in_=xg[:, j, 512:1024])
            mv = spool.tile([128, 2], F32, tag="mv", name=f"mv{ti}")
            nc.vector.bn_aggr(out=mv, in_=st)
            nc.vector.tensor_copy(out=mur_all[:, ti:ti + 1], in_=mv[:, 0:1])
            std = spool.tile([128, 1], F32, tag="std", name=f"std{ti}")
            nc.scalar.activation(out=std, in_=mv[:, 1:2], func=Act.Sqrt,
                                 bias=eps_sb, scale=1.0)
            rstd = spool.tile([128, 1], F32, tag="rstd", name=f"rstd{ti}")
            nc.vector.reciprocal(out=rstd, in_=std)
            nc.vector.tensor_copy(out=mur_all[:, 32 + ti:33 + ti], in_=rstd)

    # PE transpose [mu|r] -> rows; stage to DRAM; broadcast back
    nc.tensor.ldweights(id_sb[0:1, 0:2])      # absorb id DMA sem on PE
    murps = psum.tile([64, 128], BF16, tag="ctxA")
    nc.tensor.matmul(murps, lhsT=mur_all, rhs=id_sb, start=True, stop=True,
                     is_transpose=True)
    mur_rows = spool.tile([64, 128], BF16, tag="mur_rows")
    nc.vector.tensor_copy(out=mur_rows, in_=murps)
    nc.gpsimd.dma_start(out=mur_d[:, :], in_=mur_rows)
    MU = const.tile([128, T], BF16, tag="qT")
    R = const.tile([128, T], BF16, tag="kT")
    for q in range(4):
        nc.gpsimd.dma_start(out=MU[:, q * 1024:(q + 1) * 1024],
                            in_=bass.AP(tensor=mur_d, offset=q * 1024,
                                        ap=[[0, 128], [1, 1024]]))
        nc.gpsimd.dma_start(out=R[:, q * 1024:(q + 1) * 1024],
                            in_=bass.AP(tensor=mur_d, offset=T + q * 1024,
                                        ap=[[0, 128], [1, 1024]]))

    # zT = (xT - MU) * R in place, per d-chunk and half for pipelining
    for dc in range(8):
        for hh in range(2):
            sl = slice(hh * 2048, (hh + 1) * 2048)
            nc.vector.tensor_sub(out=xT[:, dc, sl], in0=xT[:, dc, sl], in1=MU[:, sl])
            nc.vector.tensor_mul(out=xT[:, dc, sl], in0=xT[:, dc, sl], in1=R[:, sl])
    zT = xT

    # ---- Phase 2: QKV projections ----
    for dc in range(8):  # absorb weight-load DMA sems on PE
        nc.tensor.ldweights(wq_sb[0:1, dc, 0:2])
        nc.tensor.ldweights(wk_sb[0:1, dc, 0:2])
        nc.tensor.ldweights(wv_sb[0:1, dc, 0:2])
    nc.tensor.ldweights(wo_sb[0:1, 0:2])
    qT = const.tile([128, T], BF16, tag="qT")
    kT = const.tile([128, T], BF16, tag="kT")
    v_sb = const.tile([128, 32, 130], BF16, tag="v")
    nc.vector.memset(v_sb[:, :, 64:65], 1.0)
    nc.vector.memset(v_sb[:, :, 129:130], 1.0)

    for tsl in range(8):  # 512-token slices
        cols = slice(tsl * 512, (tsl + 1) * 512)
        for name, wsb, bsb, dst in (("q", wq_sb, bq_sb, qT), ("k", wk_sb, bk_sb, kT)):
            ab = None
            if tsl > 0:
                ab = nc.tensor.ldweights(dst[0:1, (tsl - 1) * 512:(tsl - 1) * 512 + 2])
            ps = psum.tile([128, 512], F32, tag="sA" if name == "q" else "sB",
                           name=f"ps{name}{tsl}")
            for dc in range(8):
                mm = nc.tensor.matmul(ps, lhsT=wsb[:, dc, :], rhs=zT[:, dc, cols],
                                      start=(dc == 0), stop=(dc == 7))
                if dc == 0 and ab is not None:
                    after(mm, ab)
            nc.vector.tensor_scalar(out=dst[:, cols], in0=ps, scalar1=bsb,
                                    scalar2=None, op0=Alu.add)
    for ck in range(32):  # V: 128-token chunks, natural layout
        tok = slice(ck * 128, (ck + 1) * 128)
        ab = None
        if ck >= 2:
            ab = nc.tensor.ldweights(v_sb[0:1, ck - 2, 0:2])
        psv = psum.tile([128, 128], F32, tag="ctxA" if ck % 2 == 0 else "ctxB",
                        name=f"psv{ck}")
        for dc in range(8):
            mm = nc.tensor.matmul(psv, lhsT=zT[:, dc, tok], rhs=wv_sb[:, dc, :],
                                  start=(dc == 0), stop=(dc == 7))
            if dc == 0 and ab is not None:
                after(mm, ab)
        nc.vector.tensor_copy(out=v_sb[:, ck, 0:64], in_=psv[:, 0:64])
        nc.vector.tensor_copy(out=v_sb[:, ck, 65:129], in_=psv[:, 64:128])

    # ---- Phase 3: attention + normalized out_proj, partials to DRAM ----
    nc.tensor.ldweights(qT[0:1, T - 2:T])
    nc.tensor.ldweights(kT[0:1, T - 2:T])
    nc.tensor.ldweights(v_sb[0:1, 30, 0:2])
    nc.tensor.ldweights(v_sb[0:1, 31, 0:2])
    ctxT = const.tile([128, T], BF16, tag="ctxT")

    for b in range(B):
        for qt in range(NQT):
            seg = b * NQT + qt  # 0..3
            qcols = slice(b * S + qt * QT, b * S + (qt + 1) * QT)
            ctxps = {}
            for h, tag in ((0, "ctxA"), (1, "ctxB")):
                ctxps[h] = psum.tile([65, QT], F32, tag=tag, name=f"ctx{seg}h{h}")
            for kb in range(NKB):
                ck = b * NKB + kb
                kcols = slice(b * S + kb * 128, b * S + (kb + 1) * 128)
                for h, stag in ((0, "sA"), (1, "sB")):
                    hp = slice(64 * h, 64 * (h + 1))
                    sps = psum.tile([128, QT], F32, tag=stag, name=f"s{seg}k{kb}h{h}")
                    for half in range(2):
                        qh = slice(qcols.start + half * 512,
                                   qcols.start + (half + 1) * 512)
                        nc.tensor.matmul(sps[:, half * 512:(half + 1) * 512],
                                         lhsT=kT[hp, kcols], rhs=qT[hp, qh],
                                         start=True, stop=True)
                    es = espool.tile([128, QT], BF16, tag=f"es{h}",
                                     name=f"es{seg}k{kb}h{h}")
                    nc.scalar.activation(out=es, in_=sps, func=Act.Exp,
                                         bias=ebias, scale=0.125)
                    for half in range(2):
                        hs = slice(half * 512, (half + 1) * 512)
                        nc.tensor.matmul(ctxps[h][:, hs],
                                         lhsT=v_sb[:, ck, h * 65:(h + 1) * 65],
                                         rhs=es[:, hs],
                                         start=(kb == 0), stop=(kb == NKB - 1))
            # drain ctx; build per-token 1/colsum scalars (transposed via PE)
            rcpb = [espool.tile([1, QT], BF16, tag=f"rcpb{h}", bufs=2,
                                name=f"rcpb{seg}h{h}") for h in range(2)]
            for h in range(2):
                hp = slice(64 * h, 64 * (h + 1))
                nc.vector.tensor_copy(out=ctxT[hp, qcols], in_=ctxps[h][0:64, :])
                with nc.allow_low_precision(reason="bf16 softmax scale ok at 2e-2 tol"):
                    nc.vector.reciprocal(out=rcpb[h], in_=ctxps[h][64:65, :])
            rcf = espool.tile([128, 8, 2], F32, tag="rcf", bufs=2, name=f"rcf{seg}")
            for j in range(8):
                # bf16 PSUM writes must be 4B-aligned: head cols at 0 and 2
                pst = psum.tile([128, 4], BF16, tag="sA", name=f"pst{seg}j{j}")
                for h in range(2):
                    nc.tensor.matmul(pst[:, 2 * h:2 * h + 1],
                                     lhsT=rcpb[h][0:1, j * 128:(j + 1) * 128],
                                     rhs=id_sb[0:1, 0:1], start=True, stop=True,
                                     is_transpose=True)
                    nc.vector.tensor_copy(out=rcf[:, j, h:h + 1],
                                          in_=pst[:, 2 * h:2 * h + 1])
            # out_proj for this segment, overlapped with the next segment's
            # attention: reuse the just-drained ctx psum slots
            for j in range(8):
                blk = (qcols.start // 128) + j
                tok = slice(blk * 128, (blk + 1) * 128)
                osum = opool.tile([128, D], F32, tag="osum", name=f"osum{blk}")
                for h, tagps in ((0, "ctxA"), (1, "ctxB")):
                    hp = slice(64 * h, 64 * (h + 1))
                    ops = psum.tile([128, D], F32, tag=tagps, name=f"ops{blk}h{h}")
                    for half in range(2):
                        ocols = slice(half * 512, (half + 1) * 512)
                        nc.tensor.matmul(ops[:, ocols], lhsT=ctxT[hp, tok],
                                         rhs=wo_sb[hp, ocols], start=True, stop=True)
                    if h == 0:
                        nc.vector.tensor_scalar(out=osum, in0=ops,
                                                scalar1=rcf[:, j, 0:1],
                                                scalar2=None, op0=Alu.mult)
                    else:
                        nc.vector.scalar_tensor_tensor(
                            out=osum, in0=ops, scalar=rcf[:, j, 1:2], in1=osum,
                            op0=Alu.mult, op1=Alu.add)
                of = fence(osum[0:1, 0:2], 4)
                after(nc.gpsimd.dma_start(out=part_db[tok, :], in_=osum), of)

    # ---- Phase 4: cross-core reduce, downcast, output ----
    nc.gpsimd.collective_compute(
        "ReduceScatter", Alu.add, replica_groups=GROUPS,
        ins=[part_db.opt()], outs=[rs_db.opt()])
    for j in range(4):
        tok = slice(j * 128, (j + 1) * 128)
        # reuse the (dead by now) LN x-load slots for the fp32->bf16 downcast
        rsb = xpool.tile([128, D], F32, tag="xt", bufs=8, name=f"rsb{j}")
        nc.gpsimd.dma_start(out=rsb, in_=rs_db[tok, :])
        ob = xpool.tile([128, D], BF16, tag="xt", bufs=8, name=f"ob{j}")
        nc.vector.tensor_copy(out=ob, in_=rsb)
        bf = fence(ob[0:1, 0:2], 6)
        after(nc.gpsimd.dma_start(out=out_d[tok, :], in_=ob), bf)

    ctx.close()


def kernel(x, Wq, bq, Wk, bk, Wv, bv, Wo, bo, ln_gamma, ln_beta):
    x = np.asarray(x, np.float32)
    Wq, Wk, Wv, Wo = (np.asarray(w, np.float32) for w in (Wq, Wk, Wv, Wo))
    bq, bk, bv, bo = (np.asarray(v, np.float32) for v in (bq, bk, bv, bo))
    g, be = np.asarray(ln_gamma, np.float32), np.asarray(ln_beta, np.float32)

    x2 = x.reshape(T, D)
    xbf = x2.astype(BF)
    Wq_e, Wk_e, Wv_e = g[:, None] * Wq, g[:, None] * Wk, g[:, None] * Wv
    bq_e, bk_e = be @ Wq + bq, be @ Wk + bk
    host_const = (bo + (be @ Wv + bv) @ Wo).astype(np.float32)
    ident = np.eye(128, dtype=np.float32).astype(BF)

    nc = _build_nc()
    in_maps = []
    for c in range(N_CORES):
        sl = slice(128 * c, 128 * (c + 1))
        in_maps.append({
            "xsl": xbf[TS * c:TS * (c + 1)],
            "wq": Wq_e[:, sl].astype(BF),
            "wk": Wk_e[:, sl].astype(BF),
            "wv": Wv_e[:, sl].astype(BF),
            "wo": Wo[sl, :].astype(BF),
            "bq": bq_e[sl].reshape(DH, 1).astype(np.float32),
            "bk": bk_e[sl].reshape(DH, 1).astype(np.float32),
            "ident": ident,
        })
    res = run_bass_kernel_spmd(nc, in_maps, list(range(N_CORES))).results

    acc = np.concatenate([res[c]["out"] for c in range(N_CORES)],
                         axis=0).astype(np.float32)
    out = x2 + acc
    if host_const.any():
        out += host_const[None, :]
    return out.reshape(B, S, D).astype(np.float32)
